# revision 13
# baseline (speedup 1.0000x reference)
"""Multi-head causal attention (RoPE) on 8 Trainium2 NeuronCores.

Sharding (Megatron-style): core c handles batch c//4 and the 4 heads
[4*(c%4), 4*(c%4)+4). Each core computes Q/K/V projections for its
head slice, rotary embedding, causal flash-style attention (no
max-subtraction: scores are O(10) so exp is safe in f32), and its
partial output projection through the matching Wo column block. The
host sums the 4 partial outputs per batch and transposes (the device
computes out.T: [model_dim, seq]).

All on-device layouts are transposed ([feature, seq]) so that
- projections use hsT tiles as the moving operand (N=512 matmuls),
- QK^T produces scores.T directly ([key, query]) which is what the
  AV matmul wants as its moving operand, and
- softmax normalization uses a ones-matmul partition-sum that also
  broadcasts the denominator across partitions.
Matmul inputs are bf16 (f32 PE matmul is 2x slower); accumulation is
always f32 in PSUM.

Changes vs the 382us baseline (now ~333us):
- fine-grained input DMA issue order + smaller PE warmup sized to the
  ~9us DMA-queue startup; era-1 is DMA-bound (~230-300 GB/s/core).
- diagonal-block narrowing: QK / exp / mask / Ps / AV restricted to
  the causally-live query range (q >= key tile start); the boundary
  subtile uses a [128,128] triangle mask on the Pool engine.
- rope: sign-folded sin table (rows 0:64 = -sin) turns rotate-half
  into 3 muls + 1 bf16 2x-mode add on DVE, with the shifted partition
  reads going through the PSUM operand.
- attention runs one chunk delayed: attn(ic=sc-1) is emitted after
  proj(sc), so the V chains of chunk sc and the delayed Wo(ic-1)
  mt-pair units act as head-boundary/mid-diagonal PE fillers while
  the ACT engine produces the exp() stream (PE ~91% busy).
- attention j-loop emits QK in pairs ahead of the AVs; Wo PSUM tiles
  alternate between the pp and misc pools so fs drain copies never
  stall the next unit's matmuls.
- output in bf16, written as [128,1024] mt-pairs (2KB DMA rows); the
  two copies of a pair run on ACT and DVE (Pool cannot touch PSUM).
"""

import os

import numpy as np
import ml_dtypes

import concourse.bass as bass
import concourse.mybir as mybir
import concourse.tile as tile
from concourse import bacc
from concourse import bass_isa
from concourse.bass_utils import run_bass_kernel_spmd

BF16 = mybir.dt.bfloat16
F16 = mybir.dt.float16
F32 = mybir.dt.float32
NPBF16 = ml_dtypes.bfloat16
NPF16 = np.float16

NCORES = 8
B = 2
S = 2048
HDIM = 2048
NH = 16
HD = 128
HPC = 4  # heads per core
CPB = 4  # cores per batch
SCW = 512  # s-chunk width
NSC = S // SCW  # 4
KT = HDIM // 128  # 16 k-tiles
NJT = S // 128  # 16 j-tiles
NMP = KT // 2  # 8 mt-pairs for the output projection
SCALE = 1.0 / np.sqrt(HD)
ROPE_BASE = 10000.0
NWARM = 36
ROWSUM_POOL = False

_NC_CACHE: dict[str, object] = {}
LAST_EXEC_TIME_NS = None


def _build(mode: str):
    """mode: 'causal' | 'full' | 'general'"""
    nc = bacc.Bacc("TRN2", target_bir_lowering=False, debug=False,
                   num_devices=NCORES)

    hst_d = nc.declare_dram_parameter("hst", [NSC, 128, KT * SCW], BF16, isOutput=False)
    wq_d = nc.declare_dram_parameter("wq", [128, HPC * KT * 128], BF16, isOutput=False)
    wk_d = nc.declare_dram_parameter("wk", [128, HPC * KT * 128], BF16, isOutput=False)
    wv_d = nc.declare_dram_parameter("wv", [128, KT * 512], BF16, isOutput=False)
    wo_d = nc.declare_dram_parameter("wo", [128, HPC * KT * 128], BF16, isOutput=False)
    cos_d = nc.declare_dram_parameter("cost", [128, S], F32, isOutput=False)
    sin_d = nc.declare_dram_parameter("sint", [128, S], F32, isOutput=False)
    bm_d = nc.declare_dram_parameter("bmask", [128, 128], F16, isOutput=False)
    if mode == "general":
        em_d = nc.declare_dram_parameter("emask", [S, S], F16, isOutput=False)
    out_d = nc.declare_dram_parameter("outT", [NMP, 128, NSC * 1024], BF16, isOutput=True)

    Exp = mybir.ActivationFunctionType.Exp

    from contextlib import ExitStack
    with tile.TileContext(nc) as tc, ExitStack() as _es:
        if True:
            wpool = _es.enter_context(tc.tile_pool(name="wpool", bufs=1))
            cpool = _es.enter_context(tc.tile_pool(name="cpool", bufs=1))
            qkvp = _es.enter_context(tc.tile_pool(name="qkv", bufs=1))
            stripp = _es.enter_context(tc.tile_pool(name="strip", bufs=2))
            ropeT = _es.enter_context(tc.tile_pool(name="ropeT", bufs=2))
            ropeU = _es.enter_context(tc.tile_pool(name="ropeU", bufs=2))
            probsp = _es.enter_context(tc.tile_pool(name="probs", bufs=4))
            psums = _es.enter_context(tc.tile_pool(name="psums", bufs=2))
            recips = _es.enter_context(tc.tile_pool(name="recips", bufs=2))
            rsump = _es.enter_context(tc.tile_pool(name="rsum", bufs=2))
            fouts = _es.enter_context(tc.tile_pool(name="fouts", bufs=3))
            emtp = _es.enter_context(tc.tile_pool(name="emt", bufs=4))
            # PSUM: pp 2 + sp 2x[128,1024] (4 banks) + av 2 = 8 banks.
            pp_pool = _es.enter_context(tc.tile_pool(name="pp", bufs=2, space="PSUM"))
            sp_pool = _es.enter_context(tc.tile_pool(name="sp", bufs=2, space="PSUM"))
            av_pool = _es.enter_context(tc.tile_pool(name="av", bufs=2, space="PSUM"))
            wq = wpool.tile([128, HPC * KT * 128], BF16, tag="wq")
            wk = wpool.tile([128, HPC * KT * 128], BF16, tag="wk")
            wv = wpool.tile([128, KT * 512], BF16, tag="wv")
            wo = wpool.tile([128, HPC * KT * 128], BF16, tag="wo")
            cosF = cpool.tile([128, S], F32, tag="cos")
            sinS = cpool.tile([128, S], F32, tag="sin")
            bm = cpool.tile([128, 128], F16, tag="bm")
            ones = cpool.tile([128, 128], F16, tag="ones")
            nc.gpsimd.memset(ones[:], 1.0)

            qT = qkvp.tile([128, HPC * S], BF16, tag="qT")
            kTt = qkvp.tile([128, HPC * S], BF16, tag="kT")
            vN = qkvp.tile([128, NJT * 512], F16, tag="vN")
            oT = qkvp.tile([128, HPC * NSC * 512], BF16, tag="oT")

            # PE warmup: dummy matmuls keep the PE busy (HAM clock at 8/8,
            # p-state ramped) until the first input DMA chunks land.
            warm = cpool.tile([128, 512], BF16, tag="warm")
            nc.gpsimd.memset(warm[:, 0:128], 0.0)
            wps = pp_pool.tile([128, SCW], F32, tag="pp")
            for _ in range(NWARM):
                nc.tensor.matmul(wps[:], warm[:, 0:128], warm[:],
                                 start=True, stop=True)

            strips = {}
            strips[0] = stripp.tile([128, KT * SCW], BF16, name="strip0", tag="strip")
            strips[1] = stripp.tile([128, KT * SCW], BF16, name="strip1", tag="strip")

            # Wo emission in half-units: each half computes one mt block
            # (4 matmuls, ~0.9us) so the PE filler granularity matches the
            # ~174ns/j deficit vs the ACT exp stream. A pair of halves
            # shares one [128,1024] bf16 fs tile and one 2KB-row DMA.
            def make_wo_unit(ic, mp, flush=False):
                state = {}

                def emit_half(half):
                    mt = 2 * mp + half
                    if half == 0:
                        state["fs"] = fouts.tile([128, 1024], BF16, name="fs")
                    if flush:
                        # final flush: sp/av rings are dead, so rotate the fp
                        # tiles across all 8 PSUM banks -- the MMs never wait
                        # on the 687ns drain copies.
                        r = mp % 4
                        if r == 0:
                            fp = pp_pool.tile([128, SCW], F32, tag="pp", name="fp")
                        elif r == 2:
                            fp = av_pool.tile([128, SCW], F32, tag="av", name="fp")
                        else:
                            if half == 0:
                                state["spw"] = sp_pool.tile(
                                    [128, 2 * SCW], F32, tag="sp", name="fpw")
                            fp = state["spw"][:, half * SCW:(half + 1) * SCW]
                    else:
                        # both halves rotate through the pp ring (2 banks);
                        # halves are dripped >=1.7us apart so the 687ns drain
                        # copies never stall the next half's matmuls.
                        fp = pp_pool.tile([128, SCW], F32, tag="pp", name="fp")
                    for h in range(HPC):
                        nc.tensor.matmul(
                            fp[:],
                            wo[:, (h * KT + mt) * 128:(h * KT + mt + 1) * 128],
                            oT[:, (h * NSC + ic) * 512:(h * NSC + ic + 1) * 512],
                            start=(h == 0), stop=(h == HPC - 1),
                        )
                    fs = state["fs"]
                    # alternate engines for the PSUM->SBUF drain
                    if half == 0:
                        nc.scalar.copy(fs[:, 0:512], fp[:])
                    else:
                        nc.vector.tensor_copy(fs[:, 512:1024], fp[:])
                        nc.sync.dma_start(
                            out_d[mp][:, ic * 1024:(ic + 1) * 1024], fs[:])

                return [lambda: emit_half(0), lambda: emit_half(1)]

            def push_wo(ic, flush=False):
                out = []
                for mp in range(NMP):
                    out.extend(make_wo_unit(ic, mp, flush=flush))
                return out

            pending = []  # (ic, mp) Wo units delayed into the next phase

            def emit_v_chain(sc_v, st):
                stv = strips[sc_v]
                vp = pp_pool.tile([128, SCW], F32, tag="pp", name="vp")
                for kt in range(KT):
                    nc.tensor.matmul(
                        vp[:],
                        stv[:, kt * SCW + st * 128: kt * SCW + (st + 1) * 128],
                        wv[:, kt * 512:(kt + 1) * 512],
                        start=(kt == 0), stop=(kt == KT - 1),
                    )
                jt = sc_v * 4 + st
                nc.scalar.copy(vN[:, jt * 512:(jt + 1) * 512], vp[:])

            def emit_attn_head(ic, h, nj):
                """j-tiles processed in PAIRS sharing a [128,1024] 2-bank sp
                tile. Full (off-diagonal) pairs get ONE 1024-wide exp + ONE
                1024-wide Ps add, halving the ~290ns/op ACT+DVE overheads;
                diagonal pairs keep per-half exp/mask with qs narrowing. Ps2
                holds two independent partial sums (one per parity), folded
                on the Pool engine in emit_norm."""
                av = av_pool.tile([128, SCW], F32, name="av")
                Ps2 = psums.tile([128, 2 * SCW], F16, name="Ps")

                def qsof(j):
                    return 128 * (j - 4 * ic) if (
                        mode == "causal" and j > 4 * ic) else 0

                def proc_half(pr, t, j, skip_exp=False):
                    # per-half exp/mask/Ps/AV (diagonal + general mode)
                    qs = qsof(j)
                    b = t * SCW
                    if not skip_exp:
                        nc.scalar.activation(pr[:, b + qs:b + SCW],
                                             sp2s[j - t][:, b + qs:b + SCW],
                                             Exp, scale=float(SCALE))
                    if mode == "causal" and j >= ic * 4:
                        # SBUF-only op: runs on the idle Pool engine.
                        nc.gpsimd.tensor_mul(pr[:, b + qs:b + qs + 128],
                                             pr[:, b + qs:b + qs + 128], bm[:])
                    elif mode == "general":
                        emt = emtp.tile([128, SCW], F16, name="emt")
                        nc.sync.dma_start(
                            emt[:],
                            em_d[j * 128:(j + 1) * 128, ic * SCW:(ic + 1) * SCW],
                        )
                        nc.vector.tensor_mul(pr[:, b:b + SCW],
                                             pr[:, b:b + SCW], emt[:])
                    if j < 2:
                        if qs:
                            # dead prefix of this Ps2 half (ic==0 only)
                            nc.gpsimd.memset(Ps2[:, b:b + qs], 0.0)
                        nc.vector.tensor_copy(Ps2[:, b + qs:b + SCW],
                                              pr[:, b + qs:b + SCW])
                    else:
                        nc.vector.tensor_add(Ps2[:, b + qs:b + SCW],
                                             Ps2[:, b + qs:b + SCW],
                                             pr[:, b + qs:b + SCW])
                    nc.tensor.matmul(
                        av[:, qs:],
                        vN[:, j * 512 + h * 128: j * 512 + (h + 1) * 128],
                        pr[:, b + qs:b + SCW],
                        start=(j == 0), stop=(j == nj - 1),
                        skip_group_check=True,
                    )

                nfull = 4 * ic if mode == "causal" else nj
                pairs = [(2 * p, 2 * p + 1 >= nfull) for p in range(nj // 2)]
                sp2s = {}
                jc = 0
                # pair-slots processed in groups of 2 (sp ring depth): all QK
                # matmuls of a group issue before its exp/AV stream.
                for g0 in range(0, len(pairs), 2):
                    grp = pairs[g0:g0 + 2]
                    for j0, diag in grp:
                        sp2 = sp_pool.tile([128, 2 * SCW], F32, name="sp")
                        for t in range(2):
                            qs = qsof(j0 + t)
                            nc.tensor.matmul(
                                sp2[:, t * SCW + qs:(t + 1) * SCW],
                                kTt[:, h * S + (j0 + t) * 128:
                                    h * S + (j0 + t + 1) * 128],
                                qT[:, h * S + ic * SCW + qs:
                                   h * S + (ic + 1) * SCW],
                                start=True, stop=True,
                            )
                        sp2s[j0] = sp2
                    for j0, diag in grp:
                        pr = probsp.tile([128, 2 * SCW], F16, name="pr")
                        if diag and mode == "causal":
                            # one exp spanning both halves' live ranges: the
                            # dead gap [SCW : SCW+qs(j1)] gets exp of stale
                            # scores (bounded, the sp ring only ever holds
                            # scores) but is never read -- Ps adds and AV
                            # matmuls stay restricted to the live ranges.
                            qs0 = qsof(j0)
                            nc.scalar.activation(pr[:, qs0:], sp2s[j0][:, qs0:],
                                                 Exp, scale=float(SCALE))
                            for t in range(2):
                                proc_half(pr, t, j0 + t, skip_exp=True)
                        elif mode == "general":
                            for t in range(2):
                                proc_half(pr, t, j0 + t)
                        else:
                            # off-diagonal: one wide exp + Ps add + two AVs
                            nc.scalar.activation(pr[:], sp2s[j0][:], Exp,
                                                 scale=float(SCALE))
                            if j0 == 0:
                                nc.vector.tensor_copy(Ps2[:], pr[:])
                            else:
                                nc.vector.tensor_add(Ps2[:], Ps2[:], pr[:])
                            for t in range(2):
                                j = j0 + t
                                nc.tensor.matmul(
                                    av[:],
                                    vN[:, j * 512 + h * 128:
                                       j * 512 + (h + 1) * 128],
                                    pr[:, t * SCW:(t + 1) * SCW],
                                    start=(j == 0), stop=(j == nj - 1),
                                    skip_group_check=True,
                                )
                        # drip one Wo half-unit per ~3 pairs: the PE otherwise
                        # drains its QK run-ahead and then paces at the slower
                        # ACT exp rate (~290ns/pair deficit).
                        jc += 2
                        if mode == "causal" and pending and jc >= 6:
                            jc -= 6
                            pending.pop(0)()
                return av, Ps2

            def emit_norm(ic, h, av, Ps2):
                # two accumulating ones-matmuls fold the Ps2 parities AND
                # partition-sum them (broadcasting the denominator): folding
                # on POOL instead costs 1159ns + slow POOL semaphores in the
                # head-boundary critical chain.
                rs = pp_pool.tile([128, SCW], F32, tag="pp", name="rs")
                nc.tensor.matmul(rs[:], ones[:], Ps2[:, 0:SCW],
                                 start=True, stop=False)
                nc.tensor.matmul(rs[:], ones[:], Ps2[:, SCW:],
                                 start=False, stop=True)
                rc = recips.tile([128, SCW], F32, name="rc")
                nc.vector.reciprocal_approx_fast(rc[:], rs[:])
                nc.vector.tensor_mul(
                    oT[:, (h * NSC + ic) * 512:(h * NSC + ic + 1) * 512],
                    av[:], rc[:],
                )

            for sc in range(NSC):
                strip = strips[sc]
                if sc == 0:
                    # DMA issue order == consumption order. The DMA engines
                    # sustain ~230 GB/s, so the first projections are
                    # DMA-paced; everything later stays ahead.
                    nc.sync.dma_start(strip[:, 0:2048], hst_d[0][:, 0:2048])
                    nc.sync.dma_start(wq[:, 0:2048], wq_d[:, 0:2048])
                    nc.sync.dma_start(wk[:, 0:2048], wk_d[:, 0:2048])
                    for c4 in range(1, 4):
                        nc.sync.dma_start(strip[:, c4 * 2048:(c4 + 1) * 2048],
                                          hst_d[0][:, c4 * 2048:(c4 + 1) * 2048])
                    nc.sync.dma_start(cosF[:, 0:512], cos_d[:, 0:512])
                    nc.sync.dma_start(sinS[:, 0:512], sin_d[:, 0:512])
                    nc.sync.dma_start(bm[:], bm_d[:])
                    nc.sync.dma_start(wq[:, 2048:8192], wq_d[:, 2048:8192])
                    nc.sync.dma_start(wk[:, 2048:8192], wk_d[:, 2048:8192])
                    for c4 in range(4):
                        nc.sync.dma_start(wv[:, c4 * 2048:(c4 + 1) * 2048],
                                          wv_d[:, c4 * 2048:(c4 + 1) * 2048])
                    nc.sync.dma_start(strips[1][:], hst_d[1])
                    nc.sync.dma_start(cosF[:, 512:2048], cos_d[:, 512:2048])
                    nc.sync.dma_start(sinS[:, 512:2048], sin_d[:, 512:2048])
                    nc.sync.dma_start(wo[:], wo_d[:])
                elif sc + 1 < NSC:
                    strips[sc + 1] = stripp.tile([128, KT * SCW], BF16, name=f"strip{sc+1}", tag="strip")
                    nc.sync.dma_start(strips[sc + 1][:], hst_d[sc + 1])

                cs = cosF[:, sc * SCW:(sc + 1) * SCW]
                sn = sinS[:, sc * SCW:(sc + 1) * SCW]
                for h in range(HPC):
                    for wt, dst in ((wq, qT), (wk, kTt)):
                        pq = pp_pool.tile([128, SCW], F32, tag="pp")
                        for kt in range(KT):
                            nc.tensor.matmul(
                                pq[:],
                                wt[:, (h * KT + kt) * 128:(h * KT + kt + 1) * 128],
                                strip[:, kt * SCW:(kt + 1) * SCW],
                                start=(kt == 0), stop=(kt == KT - 1),
                            )
                        # rope: out = pq*cosF + rot(pq)*sinS, where sinS has
                        # the rotate-half sign folded in (rows 0:64 = -sin),
                        # so no subtract op is needed. The shifted-partition
                        # reads go through the PSUM operand (walrus only
                        # requires equal base partitions when both inputs are
                        # SBUF); the final add is bf16 SBUF = DVE 2x mode.
                        t1 = ropeT.tile([128, SCW], BF16)
                        t2 = ropeU.tile([128, SCW], BF16)
                        nc.vector.tensor_mul(t1[:], pq[:], cs)
                        nc.vector.tensor_mul(t2[0:64, :], pq[64:128, :], sn[0:64, :])
                        nc.vector.tensor_mul(t2[64:128, :], pq[0:64, :], sn[64:128, :])
                        nc.vector.tensor_add(
                            dst[:, h * S + sc * SCW: h * S + (sc + 1) * SCW],
                            t1[:], t2[:])
                if mode == "causal":
                    # Attention runs one chunk delayed: attn(ic=sc-1) is
                    # emitted after proj(sc), with the V chains of chunk sc
                    # and delayed Wo(ic-1) units as head-boundary PE fillers
                    # while the ACT engine produces the exp() stream.
                    if sc == 0:
                        for st in range(4):
                            emit_v_chain(0, st)
                    else:
                        ic = sc - 1
                        for h in range(HPC):
                            av, Ps = emit_attn_head(ic, h, 4 * (ic + 1))
                            emit_v_chain(sc, h)
                            if pending:
                                pending.pop(0)()
                            emit_norm(ic, h, av, Ps)
                        while pending:
                            pending.pop(0)()
                        pending = push_wo(ic)
                else:
                    for st in range(4):
                        emit_v_chain(sc, st)
                    ics = list(range(NSC)) if sc == NSC - 1 else []
                    for ic in ics:
                        for h in range(HPC):
                            av, Ps = emit_attn_head(ic, h, NJT)
                            for _ in range(2):
                                if pending:
                                    pending.pop(0)()
                            emit_norm(ic, h, av, Ps)
                        while pending:
                            pending.pop(0)()
                        pending = push_wo(ic)
                        if ic == NSC - 1:
                            while pending:
                                pending.pop(0)()

            if mode == "causal":
                # last attention chunk: Wo(ic2) units fill the boundaries;
                # for the last head the normalization chain is started
                # before the fillers so oT(h3) is ready when the final
                # Wo(ic3) flush begins.
                ic = NSC - 1
                for h in range(HPC):
                    av, Ps = emit_attn_head(ic, h, NJT)
                    if h == HPC - 1:
                        emit_norm(ic, h, av, Ps)
                        for _ in range(2):
                            if pending:
                                pending.pop(0)()
                    else:
                        for _ in range(2):
                            if pending:
                                pending.pop(0)()
                        emit_norm(ic, h, av, Ps)
                while pending:
                    pending.pop(0)()
                for fn in push_wo(ic, flush=True):
                    fn()

    nc.compile()
    return nc


def _get_nc(mode: str):
    if mode not in _NC_CACHE:
        _NC_CACHE[mode] = _build(mode)
    return _NC_CACHE[mode]


def _classify_mask(m: np.ndarray) -> str:
    if not m.any():
        return "full"
    tril = np.tril(np.ones((S, S), dtype=bool))
    if np.all(m[tril] == 0.0) and np.all(m[~tril] <= -1e8):
        return "causal"
    return "general"


def kernel(hidden_states, attention_mask, position_ids, Wq, Wk, Wv, Wo):
    global LAST_EXEC_TIME_NS
    hs = np.asarray(hidden_states, dtype=np.float32)
    mask = np.asarray(attention_mask, dtype=np.float32)[0, 0]
    pos = np.asarray(position_ids)
    Wq = np.asarray(Wq, dtype=np.float32)
    Wk = np.asarray(Wk, dtype=np.float32)
    Wv = np.asarray(Wv, dtype=np.float32)
    Wo = np.asarray(Wo, dtype=np.float32)

    mode = _classify_mask(mask)
    nc = _get_nc(mode)

    # rope tables per batch, [128, S] f32:
    #   cosF: the 64-row angle table duplicated to 128 rows
    #   sinS: rows 0:64 = -sin (rotate-half sign folded in), 64:128 = +sin
    inv_freq = 1.0 / (ROPE_BASE ** (np.arange(0, HD, 2, dtype=np.float32) / HD))
    cos_b, sin_b = [], []
    for b in range(B):
        ang = np.outer(pos[b].astype(np.float32), inv_freq)  # [S, 64]
        c64 = np.cos(ang).T.astype(np.float32)  # [64, S]
        s64 = np.sin(ang).T.astype(np.float32)
        cos_b.append(np.concatenate([c64, c64], axis=0).copy())
        sin_b.append(np.concatenate([-s64, s64], axis=0).copy())

    # boundary mask [128, 128]: bm[p, c] = 1 if p <= c (keep key <= query)
    pidx = np.arange(128)[:, None]
    cidx = np.arange(128)[None, :]
    bmask = (pidx <= cidx).astype(NPF16)

    emask = None
    if mode == "general":
        with np.errstate(under="ignore", over="ignore"):
            emask = np.exp(mask.T.astype(np.float64)).astype(NPF16)

    in_maps = []
    for c in range(NCORES):
        b = c // CPB
        r0 = (c % CPB) * HPC * HD  # feature-row base of this core's heads

        hsb = hs[b]  # [S, HDIM]
        hst = (hsb.reshape(NSC, SCW, KT, 128).transpose(0, 3, 2, 1)
               .reshape(NSC, 128, KT * SCW).astype(NPBF16))

        Wq_s = Wq[r0:r0 + 512]  # [512, HDIM]
        wq_t = (Wq_s.reshape(HPC, 128, KT, 128).transpose(3, 0, 2, 1)
                .reshape(128, HPC * KT * 128).astype(NPBF16))
        Wk_s = Wk[r0:r0 + 512]
        wk_t = (Wk_s.reshape(HPC, 128, KT, 128).transpose(3, 0, 2, 1)
                .reshape(128, HPC * KT * 128).astype(NPBF16))
        Wv_s = Wv[r0:r0 + 512]  # [512, HDIM]
        wv_t = (Wv_s.reshape(512, KT, 128).transpose(2, 1, 0)
                .reshape(128, KT * 512).astype(NPBF16))
        Wo_s = Wo[:, r0:r0 + 512]  # [HDIM, 512]
        wo_t = (Wo_s.reshape(KT, 128, HPC, 128).transpose(3, 2, 0, 1)
                .reshape(128, HPC * KT * 128).astype(NPBF16))

        m = {
            "hst": hst, "wq": wq_t, "wk": wk_t, "wv": wv_t, "wo": wo_t,
            "cost": cos_b[b], "sint": sin_b[b], "bmask": bmask,
        }
        if mode == "general":
            m["emask"] = emask
        in_maps.append(m)

    trace = os.environ.get("BASS_KERNEL_TRACE") == "1"
    res = run_bass_kernel_spmd(nc, in_maps, core_ids=list(range(NCORES)),
                               trace=trace)
    LAST_EXEC_TIME_NS = res.exec_time_ns

    out = np.empty((B, S, HDIM), dtype=np.float32)
    for b in range(B):
        acc = res.results[CPB * b]["outT"].astype(np.float32)
        for c in range(CPB * b + 1, CPB * (b + 1)):
            acc = acc + res.results[c]["outT"].astype(np.float32)
        # [NMP, 128, NSC*1024] -> outT [feature, query] -> out [query, feature]
        acc = (acc.reshape(NMP, 128, NSC, 2, 512).transpose(0, 3, 1, 2, 4)
               .reshape(HDIM, S))
        out[b] = acc.T
    return out

